# revision 1
# baseline (speedup 1.0000x reference)
"""Trainium2 Bass kernel for per-bag softmax attention pooling.

Problem: x [100000, 768] f32, attention_query [100000, 3] int, scope =
arange(12501)*8 (uniform bags of 8 consecutive sentences), attention_matrix
[130, 768] f32.

    att = attention_matrix[attention_query]          # [N, 3, 768]
    logits = einsum('nd,nld->nl', x, att)            # [N, 3]
    w = softmax(logits within each bag of 8)         # [N, 3]
    out[l, b, :] = sum_{n in bag b} w[n, l] * x[n]   # [3, 12500, 768]

Data-parallel over bags: 8 cores x 1568 bags (12544 sentences) each, padded
with zero bags from 12500 to 12544 total.

Per 128-sentence tile (= 16 bags):
  - PE transposes x into xT (d on partitions), then y = x @ A.T  [128, 130]
    via 6 accumulating fp32r matmuls with A.T chunks streaming.
  - logit_l = sum_g (iota==q_l) * y  -- one fused DVE scalar_tensor_tensor
    per layer (mask generation + multiply + reduce in one instruction).
  - e = exp(logits) on ACT.  WblockT[n, 16l+b] = e[n,l] * (n//8 == b).
  - One matmul [48, 769] = WblockT.T @ [x | ones] computes all 48 weighted
    sums AND the per-(layer,bag) softmax denominators (column 768) at once.
  - normalize by reciprocal of the sums column, DMA out.
"""

import json
import os

import numpy as np

import concourse.bass as bass
import concourse.mybir as mybir
from concourse.bass_utils import run_bass_kernel_spmd
from concourse.tile import TileContext

# ---------------------------------------------------------------------------
# walrus codegen in this container accepts only ONE sync-wait command per
# instruction (CTRL, S3_LW, ... structs), but Tile's add_sem_waits freely
# attaches one wait per producer proc. Post-process the serialized BIR:
# hoist excess waits onto standalone EventSemaphore instructions (the same
# thing bass's wait_ge emits) inserted right before the offender, on the
# same engine.
# ---------------------------------------------------------------------------
_orig_to_json_bytes = bass.Bass.to_json_bytes


def _to_json_bytes_split_waits(self, *args, **kwargs):
    raw = _orig_to_json_bytes(self, *args, **kwargs)
    bir = json.loads(raw)
    ctr = 0
    changed = False
    for fn in bir.get("functions", []):
        for bb in fn.get("blocks", []):
            insts = bb.get("instructions", [])
            out = []
            for inst in insts:
                si = inst.get("sync_info")
                ow = (si or {}).get("on_wait") or []
                if len(ow) > 1:
                    changed = True
                    for w in ow[:-1]:
                        ctr += 1
                        out.append(
                            {
                                "debug": inst.get("debug"),
                                "engine": inst["engine"],
                                "ins": [],
                                "name": f"I-splitw{ctr}",
                                "opcode": "EventSemaphore",
                                "outs": [],
                                "sync_info": {"on_update": [], "on_wait": [w]},
                            }
                        )
                    si["on_wait"] = [ow[-1]]
                out.append(inst)
            bb["instructions"] = out
    if not changed:
        return raw
    return json.dumps(bir).encode()


bass.Bass.to_json_bytes = _to_json_bytes_split_waits

# ---------------------------------------------------------------------------
# Problem constants (hardcoded; kernel.py must be self-contained).
# ---------------------------------------------------------------------------
N = 100000          # sentences
D = 768             # relation dim
G = 130             # classes
SEG = 8             # sentences per bag
B = N // SEG        # 12500 bags
NCORES = 8
P = 128             # partitions / sentences per tile
BAGS_PER_TILE = P // SEG            # 16
DCHUNKS = D // P                    # 6
NT = 98                             # tiles per core
ROWS_CORE = NT * P                  # 12544 sentences per core
BAGS_CORE = ROWS_CORE // SEG        # 1568 bags per core
N_PAD = ROWS_CORE * NCORES          # 100352
F32 = mybir.dt.float32
F32R = mybir.dt.float32r
# x is padded with a ones column (768) plus one zero column (769): fp32r
# matmuls require an even rhs free-dim width.
XCOLS = D + 2

LAST_EXEC_TIME_NS = None
LAST_TRACE_PATH = None


def build_nc(n_tiles=NT, passes=1):
    nc = bass.Bass("TRN2", target_bir_lowering=False)

    # x carries a ones column (col 768), added host-side, so the phase-2
    # matmul's rhs [x | 1] is a single contiguous DMA. x/at are declared
    # float32r (same bits as f32) because they feed fp32r matmuls and the
    # BIR verifier requires producers of fp32r-matmul inputs to be f32r.
    x_in = nc.dram_tensor("x", [n_tiles * P, XCOLS], F32R, kind="ExternalInput")
    q_in = nc.dram_tensor("q", [P, 3 * n_tiles], F32, kind="ExternalInput")
    at_in = nc.dram_tensor("at", [P, G * DCHUNKS], F32R, kind="ExternalInput")
    id_in = nc.dram_tensor("ident", [P, P], F32R, kind="ExternalInput")
    iota_in = nc.dram_tensor("iota", [P, G], F32, kind="ExternalInput")
    mask_in = nc.dram_tensor("mask16", [P, BAGS_PER_TILE], F32, kind="ExternalInput")
    out = nc.dram_tensor(
        "out", [3 * n_tiles * BAGS_PER_TILE, D], F32, kind="ExternalOutput"
    )
    bags_total = n_tiles * BAGS_PER_TILE

    eq = mybir.AluOpType.is_equal
    mult = mybir.AluOpType.mult
    add = mybir.AluOpType.add
    NB = BAGS_PER_TILE  # 16
    NL3 = 3 * NB        # 48

    with TileContext(nc) as tc:
        with (
            tc.tile_pool(name="const", bufs=1) as cpool,
            tc.tile_pool(name="sbuf", bufs=3) as pool,
            tc.tile_pool(name="sbxz", bufs=12) as pxz,
            tc.tile_pool(name="sbxts", bufs=5) as pxts,
            tc.tile_pool(name="sbmid", bufs=4) as pmid,
            tc.tile_pool(
                name="ps1",
                bufs=2 if os.environ.get("KERNEL_XTP2") else 1,
                space="PSUM",
            ) as ppool1,
            tc.tile_pool(name="ps2", bufs=2, space="PSUM") as ppool2,
            tc.tile_pool(
                name="ps3",
                bufs=1 if os.environ.get("KERNEL_XTP2") else 2,
                space="PSUM",
            ) as ppool3,
        ):
            id_sb = cpool.tile([P, P], F32R, tag="ident")
            nc.sync.dma_start(id_sb[:, :], id_in[:, :])
            at_sb = cpool.tile([P, G * DCHUNKS], F32R, tag="at")
            nc.sync.dma_start(at_sb[:, :], at_in[:, :])
            iota_sb = cpool.tile([P, G], F32, tag="iota")
            nc.sync.dma_start(iota_sb[:, :], iota_in[:, :])
            mask_sb = cpool.tile([P, NB], F32, tag="mask16")
            nc.sync.dma_start(mask_sb[:, :], mask_in[:, :])
            q_sb = cpool.tile([P, 3 * n_tiles], F32, tag="q")
            nc.sync.dma_start(q_sb[:, :], q_in[:, :])

            # Deep software pipeline. At iteration i the kernel emits:
            #   L(i+2):  x-tile DMA issue (SP queue = loads only)
            #   C2(i-7): reciprocal + normalize + output store (ACT queue)
            #   C1(i-6): phase-2 matmuls
            #   B2(i-4): exp + weight build
            #   B1(i-3): y-matmuls + fused logit extraction
            #   A(i):    transposes + PSUM->SBUF copy (split DVE/ACT)
            # Cross-engine data dependencies are >= 1 iteration apart, so
            # the in-order engine queues run nearly stall-free; emission
            # order doubles as scheduler priority (oldest inputs first).

            def stageL(t):
                xz = pxz.tile([P, XCOLS], F32R, tag="xz")
                nc.sync.dma_start(xz[:, :], x_in[t * P : (t + 1) * P, :])
                return xz

            dbl = os.environ.get("KERNEL_DOUBLE", "")

            def stageA(t, xz):
                xtp = ppool1.tile([P, D], F32R, tag="xtp")
                for rr in range(2 if dbl == "T" else 1):
                    for j in range(6):
                        nc.tensor.transpose(
                            xtp[:, j * P : (j + 1) * P],
                            xz[:, j * P : (j + 1) * P],
                            id_sb[:, :],
                        )
                xts = pxts.tile([P, D], F32R, tag="xts")
                cm = os.environ.get("KERNEL_COPYMODE", "split")
                for rr in range(2 if dbl == "COPY" else 1):
                    if cm == "act":
                        nc.scalar.copy(xts[:, :], xtp[:, :])
                    elif cm == "dve":
                        nc.vector.tensor_copy(xts[:, :], xtp[:, :])
                    elif cm == "split512":
                        nc.vector.tensor_copy(xts[:, 0:256], xtp[:, 0:256])
                        nc.scalar.copy(xts[:, 256:D], xtp[:, 256:D])
                    else:
                        nc.vector.tensor_copy(xts[:, 0:384], xtp[:, 0:384])
                        nc.scalar.copy(xts[:, 384:D], xtp[:, 384:D])
                return xts

            def stageB1(t, xts):
                yp = ppool2.tile([P, G], F32, tag="yp")
                nreps = 2 if dbl == "Y" else 1
                for rr in range(nreps):
                    for j in range(6):
                        nc.tensor.matmul(
                            yp[:, :],
                            xts[:, j * P : (j + 1) * P],
                            at_sb[:, j * G : (j + 1) * G],
                            start=(rr == 0 and j == 0),
                            stop=(rr == nreps - 1 and j == 5),
                        )
                logit = pool.tile([P, 3], F32, tag="logit")
                scratch = pool.tile([P, G], F32, tag="scratch")
                for rr in range(2 if dbl == "STT" else 1):
                    for layer in range(3):
                        nc.vector.scalar_tensor_tensor(
                            scratch[:, :],
                            iota_sb[:, :],
                            q_sb[:, 3 * t + layer : 3 * t + layer + 1],
                            yp[:, :],
                            op0=eq,
                            op1=mult,
                            accum_out=logit[:, layer : layer + 1],
                        )
                return logit

            def stageB2(t, logit):
                e = pool.tile([P, 3], F32, tag="e")
                nc.scalar.activation(
                    e[:, :], logit[:, :], mybir.ActivationFunctionType.Exp
                )
                wb = pmid.tile([P, NL3], F32R, tag="wb")
                for layer in range(3):
                    nc.vector.tensor_scalar_mul(
                        wb[:, layer * NB : (layer + 1) * NB],
                        mask_sb[:, :],
                        e[:, layer : layer + 1],
                    )
                return wb

            def stageC1(t, xz, wb):
                p2 = ppool3.tile([NL3, XCOLS], F32, tag="p2")
                nreps = 2 if dbl == "P2" else 1
                for rr in range(nreps):
                    st, sp = rr == 0, rr == nreps - 1
                    nc.tensor.matmul(
                        p2[:, 0:512], wb[:, :], xz[:, 0:512], start=st, stop=sp
                    )
                    nc.tensor.matmul(
                        p2[:, 512:XCOLS], wb[:, :], xz[:, 512:XCOLS],
                        start=st, stop=sp,
                    )
                return p2

            def stageC2(t, p2):
                inv_s = pool.tile([NL3, 1], F32, tag="inv_s")
                nc.vector.reciprocal(inv_s[:, :], p2[:, D : D + 1])
                outs = pool.tile([NL3, D], F32, tag="outs")
                nm = os.environ.get("KERNEL_NORM", "act")
                if nm == "dve":
                    nc.vector.tensor_scalar_mul(
                        outs[:, 0:D], p2[:, 0:D], inv_s[:, :]
                    )
                elif nm == "dve256":
                    nc.scalar.activation(
                        outs[:, 0:512],
                        p2[:, 0:512],
                        mybir.ActivationFunctionType.Copy,
                        scale=inv_s[:, :],
                    )
                    nc.vector.tensor_scalar_mul(
                        outs[:, 512:D], p2[:, 512:D], inv_s[:, :]
                    )
                else:
                    nc.scalar.activation(
                        outs[:, 0:512],
                        p2[:, 0:512],
                        mybir.ActivationFunctionType.Copy,
                        scale=inv_s[:, :],
                    )
                    nc.scalar.activation(
                        outs[:, 512:D],
                        p2[:, 512:D],
                        mybir.ActivationFunctionType.Copy,
                        scale=inv_s[:, :],
                    )
                # one contiguous store per tile; host reshuffles layers
                nc.scalar.dma_start(
                    out[t * NL3 : (t + 1) * NL3, :], outs[:, :]
                )

            for rep in range(passes):
                stL = {}
                stXts = {}
                stLogit = {}
                stWb = {}
                stP2 = {}
                for i in range(-4, n_tiles + 8):
                    if 0 <= i + 4 < n_tiles:
                        stL[i + 4] = stageL(i + 4)
                    if 0 <= i - 7 < n_tiles:
                        stageC2(i - 7, stP2.pop(i - 7))
                    if 0 <= i - 6 < n_tiles:
                        stP2[i - 6] = stageC1(
                            i - 6, stL.pop(i - 6), stWb.pop(i - 6)
                        )
                    if 0 <= i - 4 < n_tiles:
                        stWb[i - 4] = stageB2(i - 4, stLogit.pop(i - 4))
                    if 0 <= i - 3 < n_tiles:
                        stLogit[i - 3] = stageB1(i - 3, stXts[i - 3])
                    if 0 <= i < n_tiles:
                        stXts[i] = stageA(i, stL[i])
                    if 0 <= i - 3 < n_tiles:
                        del stXts[i - 3]

    return nc


# ---------------------------------------------------------------------------
# Host-side constants + sharding
# ---------------------------------------------------------------------------


def _host_constants(attention_matrix):
    a = np.ascontiguousarray(np.asarray(attention_matrix, dtype=np.float32))
    assert a.shape == (G, D)
    at = a.T  # [768, 130]
    at_r = np.ascontiguousarray(
        at.reshape(DCHUNKS, P, G).transpose(1, 0, 2).reshape(P, DCHUNKS * G)
    )
    ident = np.eye(P, dtype=np.float32)
    iota = np.tile(np.arange(G, dtype=np.float32), (P, 1))
    mask16 = (
        (np.arange(P)[:, None] // SEG) == np.arange(BAGS_PER_TILE)[None, :]
    ).astype(np.float32)
    return at_r, ident, iota, mask16


def kernel(x, attention_query, scope, attention_matrix):
    x = np.asarray(x)
    attention_query = np.asarray(attention_query)
    assert x.shape == (N, D) and attention_query.shape == (N, 3)

    at_r, ident, iota, mask16 = _host_constants(attention_matrix)

    x_pad = np.zeros((N_PAD, XCOLS), dtype=np.float32)
    x_pad[:N, :D] = x
    x_pad[:, D] = 1.0
    q_pad = np.zeros((N_PAD, 3), dtype=np.float32)
    q_pad[:N] = attention_query.astype(np.float32)

    in_maps = []
    for c in range(NCORES):
        xs = x_pad[c * ROWS_CORE : (c + 1) * ROWS_CORE]
        qs = (
            q_pad[c * ROWS_CORE : (c + 1) * ROWS_CORE]
            .reshape(NT, P, 3)
            .transpose(1, 0, 2)
            .reshape(P, 3 * NT)
        )
        in_maps.append(
            {
                "x": np.ascontiguousarray(xs),
                "q": np.ascontiguousarray(qs),
                "at": at_r,
                "ident": ident,
                "iota": iota,
                "mask16": mask16,
            }
        )

    nc = build_nc()
    trace = bool(int(os.environ.get("KERNEL_TRACE", "0")))
    res = run_bass_kernel_spmd(
        nc, in_maps, core_ids=list(range(NCORES)), trace=trace
    )
    global LAST_EXEC_TIME_NS, LAST_TRACE_PATH
    LAST_EXEC_TIME_NS = res.exec_time_ns
    if trace:
        print(f"HW exec time: {res.exec_time_ns} ns")
        if res.instructions_and_trace is not None:
            LAST_TRACE_PATH = res.instructions_and_trace[1]
            print("trace:", LAST_TRACE_PATH)

    # per-core out is [NT*48, 768]: tile-major blocks of (3 layers x 16 bags)
    parts = [
        r["out"]
        .reshape(NT, 3, BAGS_PER_TILE, D)
        .transpose(1, 0, 2, 3)
        .reshape(3, BAGS_CORE, D)
        for r in res.results
    ]
    full = np.concatenate(parts, axis=1)[:, :B, :]
    return np.ascontiguousarray(full)



# revision 8
# speedup vs baseline: 1.0340x; 1.0340x over previous
"""Trainium2 Bass kernel for per-bag softmax attention pooling (v2, fp16).

Problem: x [100000, 768] f32, attention_query [100000, 3] int, scope =
arange(12501)*8 (uniform bags of 8 consecutive sentences), attention_matrix
[130, 768] f32.

    att = attention_matrix[attention_query]          # [N, 3, 768]
    logits = einsum('nd,nld->nl', x, att)            # [N, 3]
    w = softmax(logits within each bag of 8)         # [N, 3]
    out[l, b, :] = sum_{n in bag b} w[n, l] * x[n]   # [3, 12500, 768]

Data-parallel over bags: 8 cores x 1568 bags (12544 sentences) each, padded
with zero bags from 12500 to 12544 total.

v2 design (probe-driven; the v1 kernel was DMA-bound with a partition-
starved output store):
  - x ships to DRAM as fp16 (host cast): input traffic halved. rel-err
    budget 2e-2 is ~10x above the fp16 quantization error.
  - output is written TRANSPOSED per tile: p2[p, j*48 + l*16 + b] =
    out[l, bag, j*128+p], i.e. a [128, 288] fp16 tile -> all 128 DMA
    partitions active (v1 stored [48, 768] f32: 48 partitions = ~6/16
    SDMA engines and 4x the bytes).
  - all matmuls are fp16 (1 cycle/row; fp32r at free-dim < 256 runs at
    4 cycles/row when warm, which made v1's y-matmuls 4x slower).
  - softmax weights are pre-normalized on the fly: per-bag sums come from
    one tiny matmul against a constant 128x128 block-diagonal bag-
    adjacency matrix (s_sent = bagadj @ e), then w = e * recip(s) before
    the weighted-sum matmul. This kills v1's 768-col normalize pass.

Per 128-sentence tile (= 16 bags):
  A : 6 PE transposes (fp16) -> PSUM, cast-copy to xts fp16 (DVE/ACT).
  B1: y = xT.T @ A.T  [128, 130] via 6 fp16 matmuls; 3 fused DVE
      scalar_tensor_tensor ops extract logit_l = y[n, q_l].
  B2a: e = exp(logits) (ACT, fp16 out); s_sent = bagadj @ e (PE).
  B2b: en = e * recip(s_sent) (DVE); wb[n, 16l+b] = en[n,l]*mask[n,b].
  C1: 6 fp16 matmuls p2[:, 48j:48j+48] = xz_j.T @ wb  (out transposed).
  C2: cast-copy p2 -> fp16, one 128-partition DMA store per tile.
"""

import json
import os

import numpy as np

import concourse.bass as bass
import concourse.mybir as mybir
from concourse.bass_utils import run_bass_kernel_spmd
from concourse.tile import TileContext

# ---------------------------------------------------------------------------
# walrus codegen in this container accepts only ONE sync-wait command per
# instruction (CTRL, S3_LW, ... structs), but Tile's add_sem_waits freely
# attaches one wait per producer proc. Post-process the serialized BIR:
# hoist excess waits onto standalone EventSemaphore instructions (the same
# thing bass's wait_ge emits) inserted right before the offender, on the
# same engine.
# ---------------------------------------------------------------------------
_orig_to_json_bytes = bass.Bass.to_json_bytes


def _to_json_bytes_split_waits(self, *args, **kwargs):
    raw = _orig_to_json_bytes(self, *args, **kwargs)
    bir = json.loads(raw)
    ctr = 0
    changed = False
    for fn in bir.get("functions", []):
        for bb in fn.get("blocks", []):
            insts = bb.get("instructions", [])
            out = []
            for inst in insts:
                si = inst.get("sync_info")
                ow = (si or {}).get("on_wait") or []
                if len(ow) > 1:
                    changed = True
                    for w in ow[:-1]:
                        ctr += 1
                        out.append(
                            {
                                "debug": inst.get("debug"),
                                "engine": inst["engine"],
                                "ins": [],
                                "name": f"I-splitw{ctr}",
                                "opcode": "EventSemaphore",
                                "outs": [],
                                "sync_info": {"on_update": [], "on_wait": [w]},
                            }
                        )
                    si["on_wait"] = [ow[-1]]
                out.append(inst)
            bb["instructions"] = out
    if not changed:
        return raw
    return json.dumps(bir).encode()


bass.Bass.to_json_bytes = _to_json_bytes_split_waits

# ---------------------------------------------------------------------------
# Problem constants (hardcoded; kernel.py must be self-contained).
# ---------------------------------------------------------------------------
N = 100000          # sentences
D = 768             # relation dim
G = 130             # classes
SEG = 8             # sentences per bag
B = N // SEG        # 12500 bags
NCORES = 8
P = 128             # partitions / sentences per tile
BAGS_PER_TILE = P // SEG            # 16
DCHUNKS = D // P                    # 6
NT = 98                             # tiles per core
ROWS_CORE = NT * P                  # 12544 sentences per core
BAGS_CORE = ROWS_CORE // SEG        # 1568 bags per core
N_PAD = ROWS_CORE * NCORES          # 100352
F16 = mybir.dt.float16
F32 = mybir.dt.float32
F32R = mybir.dt.float32r
NB = BAGS_PER_TILE      # 16
NL3 = 3 * NB            # 48
OUTW = DCHUNKS * NL3    # 288

LAST_EXEC_TIME_NS = None
LAST_TRACE_PATH = None


def build_nc(n_tiles=NT, passes=1):
    nc = bass.Bass("TRN2", target_bir_lowering=False)

    x_in = nc.dram_tensor("x", [n_tiles * P, D], F16, kind="ExternalInput")
    q_in = nc.dram_tensor("q", [P, 3 * n_tiles], F32, kind="ExternalInput")
    at_in = nc.dram_tensor("at", [P, G * DCHUNKS], F16, kind="ExternalInput")
    id_in = nc.dram_tensor("ident", [P, P], F16, kind="ExternalInput")
    iota_in = nc.dram_tensor("iota", [P, G], F32, kind="ExternalInput")
    mask_in = nc.dram_tensor("mask16", [P, NB], F16, kind="ExternalInput")
    adj_in = nc.dram_tensor("bagadj", [P, P], F16, kind="ExternalInput")
    out = nc.dram_tensor(
        "out", [n_tiles * P, OUTW], F16, kind="ExternalOutput"
    )

    eq = mybir.AluOpType.is_equal
    mult = mybir.AluOpType.mult
    dbl = os.environ.get("KERNEL_DOUBLE", "")
    SPLIT = int(os.environ.get("KERNEL_SPLIT", "256"))    # xts copy DVE share
    CSPL = int(os.environ.get("KERNEL_CSPL", "96"))       # C2 copy DVE share

    with TileContext(nc) as tc:
        with (
            tc.tile_pool(name="const", bufs=1) as cpool,
            tc.tile_pool(name="sbuf", bufs=3) as pool,
            tc.tile_pool(name="sbxz", bufs=13) as pxz,
            tc.tile_pool(name="sbxts", bufs=5) as pxts,
            tc.tile_pool(name="sbmid", bufs=4) as pmid,
            tc.tile_pool(name="sbout", bufs=3) as pouts,
            tc.tile_pool(name="ps1", bufs=2, space="PSUM") as ppool1,
            tc.tile_pool(name="ps2", bufs=2, space="PSUM") as ppool2,
            tc.tile_pool(name="pss", bufs=2, space="PSUM") as ppool_s,
            tc.tile_pool(name="ps3", bufs=2, space="PSUM") as ppool3,
        ):
            id_sb = cpool.tile([P, P], F16, tag="ident")
            nc.sync.dma_start(id_sb[:, :], id_in[:, :])
            at_sb = cpool.tile([P, G * DCHUNKS], F16, tag="at")
            nc.sync.dma_start(at_sb[:, :], at_in[:, :])
            iota_sb = cpool.tile([P, G], F32, tag="iota")
            nc.sync.dma_start(iota_sb[:, :], iota_in[:, :])
            mask_sb = cpool.tile([P, NB], F16, tag="mask16")
            nc.sync.dma_start(mask_sb[:, :], mask_in[:, :])
            adj_sb = cpool.tile([P, P], F16, tag="bagadj")
            nc.sync.dma_start(adj_sb[:, :], adj_in[:, :])
            q_sb = cpool.tile([P, 3 * n_tiles], F32, tag="q")
            nc.sync.dma_start(q_sb[:, :], q_in[:, :])

            # logit tiles live in a manual ring: column 3 is zeroed once and
            # never rewritten, so exp() can process [128, 4] wholesale (the
            # 4th lane keeps e=1, feeding the unused s_sent column).
            NRING = 3
            logit_ring = []
            for k in range(NRING):
                lt = cpool.tile([P, 4], F32, tag=f"logit{k}")
                nc.vector.memset(lt[:, 3:4], 0.0)
                logit_ring.append(lt)

            def stageL(t):
                xz = pxz.tile([P, D], F16, tag="xz")
                nc.sync.dma_start(xz[:, :], x_in[t * P : (t + 1) * P, :])
                if dbl == "DMAIN":
                    nc.sync.dma_start(xz[:, :], x_in[t * P : (t + 1) * P, :])
                return xz

            def stageA(t, xz):
                xtp = ppool1.tile([P, D], F16, tag="xtp")
                for rr in range(2 if dbl == "T" else 1):
                    for j in range(6):
                        nc.tensor.transpose(
                            xtp[:, j * P : (j + 1) * P],
                            xz[:, j * P : (j + 1) * P],
                            id_sb[:, :],
                        )
                xts = pxts.tile([P, D], F16, tag="xts")
                for rr in range(2 if dbl == "COPY" else 1):
                    if SPLIT > 0:
                        nc.vector.tensor_copy(xts[:, 0:SPLIT], xtp[:, 0:SPLIT])
                    if SPLIT < D:
                        nc.scalar.copy(xts[:, SPLIT:D], xtp[:, SPLIT:D])
                return xts

            def stageB1(t, xts):
                yp = ppool2.tile([P, G], F32, tag="yp")
                nreps = 2 if dbl == "Y" else 1
                for rr in range(nreps):
                    for j in range(6):
                        nc.tensor.matmul(
                            yp[:, :],
                            xts[:, j * P : (j + 1) * P],
                            at_sb[:, j * G : (j + 1) * G],
                            start=(rr == 0 and j == 0),
                            stop=(rr == nreps - 1 and j == 5),
                        )
                logit = logit_ring[t % NRING]
                scratch = pool.tile([P, G], F32, tag="scratch")
                for rr in range(2 if dbl == "STT" else 1):
                    for layer in range(3):
                        nc.vector.scalar_tensor_tensor(
                            scratch[:, :],
                            iota_sb[:, :],
                            q_sb[:, 3 * t + layer : 3 * t + layer + 1],
                            yp[:, :],
                            op0=eq,
                            op1=mult,
                            accum_out=logit[:, layer : layer + 1],
                        )
                return logit

            def stageB2a(t, logit):
                e4 = pool.tile([P, 4], F16, tag="e4")
                nc.scalar.activation(
                    e4[:, :], logit[:, :], mybir.ActivationFunctionType.Exp
                )
                ss = ppool_s.tile([P, 4], F32, tag="ss")
                nc.tensor.matmul(
                    ss[:, :], adj_sb[:, :], e4[:, :], start=True, stop=True
                )
                return e4, ss

            def stageB2b(t, e4, ss):
                rs = pool.tile([P, 4], F32, tag="rs")
                nc.vector.reciprocal(rs[:, :], ss[:, :])
                en = pool.tile([P, 4], F32, tag="en")
                nc.vector.tensor_mul(en[:, :], e4[:, :], rs[:, :])
                wb = pmid.tile([P, NL3], F16, tag="wb")
                for layer in range(3):
                    nc.vector.tensor_scalar_mul(
                        wb[:, layer * NB : (layer + 1) * NB],
                        mask_sb[:, :],
                        en[:, layer : layer + 1],
                    )
                return wb

            def stageC1(t, xz, wb):
                p2 = ppool3.tile([P, OUTW], F32, tag="p2")
                nreps = 2 if dbl == "P2" else 1
                for rr in range(nreps):
                    for j in range(6):
                        nc.tensor.matmul(
                            p2[:, j * NL3 : (j + 1) * NL3],
                            xz[:, j * P : (j + 1) * P],
                            wb[:, :],
                            start=(rr == 0),
                            stop=(rr == nreps - 1),
                        )
                return p2

            def stageC2(t, p2):
                outs = pouts.tile([P, OUTW], F16, tag="outs")
                for rr in range(2 if dbl == "C2" else 1):
                    if CSPL > 0:
                        nc.vector.tensor_copy(outs[:, 0:CSPL], p2[:, 0:CSPL])
                    if CSPL < OUTW:
                        nc.scalar.copy(outs[:, CSPL:OUTW], p2[:, CSPL:OUTW])
                nc.scalar.dma_start(
                    out[t * P : (t + 1) * P, :], outs[:, :]
                )
                if dbl == "DMAOUT":
                    nc.scalar.dma_start(
                        out[t * P : (t + 1) * P, :], outs[:, :]
                    )

            for rep in range(passes):
                stL = {}
                stXts = {}
                stLog = {}
                stES = {}
                stWb = {}
                stP2 = {}
                for i in range(-4, n_tiles + 9):
                    if 0 <= i + 4 < n_tiles:
                        stL[i + 4] = stageL(i + 4)
                    if 0 <= i - 8 < n_tiles:
                        stageC2(i - 8, stP2.pop(i - 8))
                    if 0 <= i - 7 < n_tiles:
                        stP2[i - 7] = stageC1(
                            i - 7, stL.pop(i - 7), stWb.pop(i - 7)
                        )
                    if 0 <= i - 5 < n_tiles:
                        stWb[i - 5] = stageB2b(i - 5, *stES.pop(i - 5))
                    if 0 <= i - 4 < n_tiles:
                        stES[i - 4] = stageB2a(i - 4, stLog.pop(i - 4))
                    if 0 <= i - 3 < n_tiles:
                        stLog[i - 3] = stageB1(i - 3, stXts.pop(i - 3))
                    if 0 <= i < n_tiles:
                        stXts[i] = stageA(i, stL[i])

    return nc


# ---------------------------------------------------------------------------
# Host-side constants + sharding
# ---------------------------------------------------------------------------


def _host_constants(attention_matrix):
    a = np.ascontiguousarray(np.asarray(attention_matrix, dtype=np.float32))
    assert a.shape == (G, D)
    at = a.T  # [768, 130]
    at_r = np.ascontiguousarray(
        at.reshape(DCHUNKS, P, G).transpose(1, 0, 2).reshape(P, DCHUNKS * G)
    ).astype(np.float16)
    ident = np.eye(P, dtype=np.float16)
    iota = np.tile(np.arange(G, dtype=np.float32), (P, 1))
    mask16 = (
        (np.arange(P)[:, None] // SEG) == np.arange(NB)[None, :]
    ).astype(np.float16)
    bagadj = (
        (np.arange(P)[:, None] // SEG) == (np.arange(P)[None, :] // SEG)
    ).astype(np.float16)
    return at_r, ident, iota, mask16, bagadj


def make_in_maps(x, attention_query, attention_matrix):
    at_r, ident, iota, mask16, bagadj = _host_constants(attention_matrix)

    x_pad = np.zeros((N_PAD, D), dtype=np.float16)
    x_pad[:N] = x.astype(np.float16)
    q_pad = np.zeros((N_PAD, 3), dtype=np.float32)
    q_pad[:N] = attention_query.astype(np.float32)

    in_maps = []
    for c in range(NCORES):
        xs = x_pad[c * ROWS_CORE : (c + 1) * ROWS_CORE]
        qs = (
            q_pad[c * ROWS_CORE : (c + 1) * ROWS_CORE]
            .reshape(NT, P, 3)
            .transpose(1, 0, 2)
            .reshape(P, 3 * NT)
        )
        in_maps.append(
            {
                "x": np.ascontiguousarray(xs),
                "q": np.ascontiguousarray(qs),
                "at": at_r,
                "ident": ident,
                "iota": iota,
                "mask16": mask16,
                "bagadj": bagadj,
            }
        )
    return in_maps


def kernel(x, attention_query, scope, attention_matrix):
    x = np.asarray(x)
    attention_query = np.asarray(attention_query)
    assert x.shape == (N, D) and attention_query.shape == (N, 3)

    in_maps = make_in_maps(x, attention_query, attention_matrix)

    nc = build_nc()
    trace = bool(int(os.environ.get("KERNEL_TRACE", "0")))
    res = run_bass_kernel_spmd(
        nc, in_maps, core_ids=list(range(NCORES)), trace=trace
    )
    global LAST_EXEC_TIME_NS, LAST_TRACE_PATH
    LAST_EXEC_TIME_NS = res.exec_time_ns
    if trace:
        print(f"HW exec time: {res.exec_time_ns} ns")
        if res.instructions_and_trace is not None:
            LAST_TRACE_PATH = res.instructions_and_trace[1]
            print("trace:", LAST_TRACE_PATH)

    # per-core out is [NT*128, 288] fp16 with p2[p, 48j + 16l + b] =
    # out[l, 16t + b, 128j + p]
    parts = [
        r["out"]
        .reshape(NT, P, DCHUNKS, 3, NB)
        .transpose(3, 0, 4, 2, 1)
        .reshape(3, BAGS_CORE, D)
        for r in res.results
    ]
    full = np.concatenate(parts, axis=1)[:, :B, :].astype(np.float32)
    return np.ascontiguousarray(full)


# revision 23
# speedup vs baseline: 1.0628x; 1.0278x over previous
"""Trainium2 Bass kernel for per-bag softmax attention pooling (v2, fp16).

Problem: x [100000, 768] f32, attention_query [100000, 3] int, scope =
arange(12501)*8 (uniform bags of 8 consecutive sentences), attention_matrix
[130, 768] f32.

    att = attention_matrix[attention_query]          # [N, 3, 768]
    logits = einsum('nd,nld->nl', x, att)            # [N, 3]
    w = softmax(logits within each bag of 8)         # [N, 3]
    out[l, b, :] = sum_{n in bag b} w[n, l] * x[n]   # [3, 12500, 768]

Data-parallel over bags: 8 cores x 1568 bags (12544 sentences) each, padded
with zero bags from 12500 to 12544 total.

v2 design (probe-driven; the v1 kernel was DMA-bound with a partition-
starved output store):
  - x ships to DRAM as fp16 (host cast): input traffic halved. rel-err
    budget 2e-2 is ~10x above the fp16 quantization error.
  - output is written TRANSPOSED per tile: p2[p, j*48 + l*16 + b] =
    out[l, bag, j*128+p], i.e. a [128, 288] fp16 tile -> all 128 DMA
    partitions active (v1 stored [48, 768] f32: 48 partitions = ~6/16
    SDMA engines and 4x the bytes).
  - all matmuls are fp16 (1 cycle/row; fp32r at free-dim < 256 runs at
    4 cycles/row when warm, which made v1's y-matmuls 4x slower).
  - softmax weights are pre-normalized on the fly: per-bag sums come from
    one tiny matmul against a constant 128x128 block-diagonal bag-
    adjacency matrix (s_sent = bagadj @ e), then w = e * recip(s) before
    the weighted-sum matmul. This kills v1's 768-col normalize pass.

Per 128-sentence tile (= 16 bags):
  A : 6 PE transposes (fp16) -> PSUM, cast-copy to xts fp16 (DVE/ACT).
  B1: y = xT.T @ A.T  [128, 130] via 6 fp16 matmuls; 3 fused DVE
      scalar_tensor_tensor ops extract logit_l = y[n, q_l].
  B2a: e = exp(logits) (ACT, fp16 out); s_sent = bagadj @ e (PE).
  B2b: en = e * recip(s_sent) (DVE); wb[n, 16l+b] = en[n,l]*mask[n,b].
  C1: 6 fp16 matmuls p2[:, 48j:48j+48] = xz_j.T @ wb  (out transposed).
  C2: cast-copy p2 -> fp16, one 128-partition DMA store per tile.
"""

import json
import os

import numpy as np

import concourse.bass as bass
import concourse.mybir as mybir
from concourse.bass_utils import run_bass_kernel_spmd
from concourse.tile import TileContext

# ---------------------------------------------------------------------------
# walrus codegen in this container accepts only ONE sync-wait command per
# instruction (CTRL, S3_LW, ... structs), but Tile's add_sem_waits freely
# attaches one wait per producer proc. Post-process the serialized BIR:
# hoist excess waits onto standalone EventSemaphore instructions (the same
# thing bass's wait_ge emits) inserted right before the offender, on the
# same engine.
# ---------------------------------------------------------------------------
_orig_to_json_bytes = getattr(
    bass.Bass.to_json_bytes, "_split_waits_orig", bass.Bass.to_json_bytes
)


def _to_json_bytes_split_waits(self, *args, **kwargs):
    raw = _orig_to_json_bytes(self, *args, **kwargs)
    bir = json.loads(raw)
    ctr = 0
    changed = False
    for fn in bir.get("functions", []):
        for bb in fn.get("blocks", []):
            insts = bb.get("instructions", [])
            out = []
            for inst in insts:
                si = inst.get("sync_info")
                ow = (si or {}).get("on_wait") or []
                if len(ow) > 1:
                    changed = True
                    for w in ow[:-1]:
                        ctr += 1
                        out.append(
                            {
                                "debug": inst.get("debug"),
                                "engine": inst["engine"],
                                "ins": [],
                                "name": f"I-splitw{ctr}",
                                "opcode": "EventSemaphore",
                                "outs": [],
                                "sync_info": {"on_update": [], "on_wait": [w]},
                            }
                        )
                    si["on_wait"] = [ow[-1]]
                out.append(inst)
            bb["instructions"] = out
    if not changed:
        return raw
    return json.dumps(bir).encode()


_to_json_bytes_split_waits._split_waits_orig = _orig_to_json_bytes
bass.Bass.to_json_bytes = _to_json_bytes_split_waits

# ---------------------------------------------------------------------------
# Problem constants (hardcoded; kernel.py must be self-contained).
# ---------------------------------------------------------------------------
N = 100000          # sentences
D = 768             # relation dim
G = 130             # classes
SEG = 8             # sentences per bag
B = N // SEG        # 12500 bags
NCORES = 8
P = 128             # partitions / sentences per tile
BAGS_PER_TILE = P // SEG            # 16
DCHUNKS = D // P                    # 6
NT = 98                             # tiles per core
ROWS_CORE = NT * P                  # 12544 sentences per core
BAGS_CORE = ROWS_CORE // SEG        # 1568 bags per core
N_PAD = ROWS_CORE * NCORES          # 100352
F16 = mybir.dt.float16
F32 = mybir.dt.float32
F32R = mybir.dt.float32r
NB = BAGS_PER_TILE      # 16
NL3 = 3 * NB            # 48
OUTW = DCHUNKS * NL3    # 288

LAST_EXEC_TIME_NS = None
LAST_TRACE_PATH = None


def build_nc(n_tiles=NT, passes=1):
    nc = bass.Bass("TRN2", target_bir_lowering=False)

    x_in = nc.dram_tensor("x", [n_tiles * P, D], F16, kind="ExternalInput")
    q_in = nc.dram_tensor("q", [P, 3 * n_tiles], F32, kind="ExternalInput")
    at_in = nc.dram_tensor("at", [P, G * DCHUNKS], F16, kind="ExternalInput")
    id_in = nc.dram_tensor("ident", [P, P], F16, kind="ExternalInput")
    iota_in = nc.dram_tensor("iota", [P, G], F16, kind="ExternalInput")
    mask_in = nc.dram_tensor("mask16", [P, NB], F16, kind="ExternalInput")
    adj_in = nc.dram_tensor("bagadj", [P, P], F16, kind="ExternalInput")
    # output stores are batched GROUP tiles per DMA: 576B/partition
    # descriptors hit the SDMA small-transfer penalty, GROUP*576B do not.
    GROUP = int(os.environ.get("KERNEL_OGROUP", "2"))
    assert n_tiles % GROUP == 0
    out = nc.dram_tensor(
        "out", [(n_tiles // GROUP) * P, GROUP * OUTW], F16,
        kind="ExternalOutput",
    )

    eq = mybir.AluOpType.is_equal
    mult = mybir.AluOpType.mult
    div = mybir.AluOpType.divide
    dbl = os.environ.get("KERNEL_DOUBLE", "")
    SPLIT = int(os.environ.get("KERNEL_SPLIT", "512"))    # xts copy DVE share
    CSPL = int(os.environ.get("KERNEL_CSPL", "0"))        # C2 copy DVE share
    STTENG = os.environ.get("KERNEL_STTENG", "dve")       # dve (pool: no ISA)
    DIVMODE = os.environ.get("KERNEL_DIV", "recip")       # recip (div: no ISA)

    with TileContext(nc) as tc:
        with (
            tc.tile_pool(name="const", bufs=1) as cpool,
            tc.tile_pool(name="sbuf", bufs=3) as pool,
            tc.tile_pool(name="sbxz", bufs=13) as pxz,
            tc.tile_pool(name="sbxts", bufs=5) as pxts,
            tc.tile_pool(name="sbmid", bufs=4) as pmid,
            tc.tile_pool(name="sbout", bufs=3) as pouts,
            tc.tile_pool(name="ps1", bufs=2, space="PSUM") as ppool1,
            tc.tile_pool(name="ps2", bufs=2, space="PSUM") as ppool2,
            tc.tile_pool(name="pss", bufs=2, space="PSUM") as ppool_s,
            tc.tile_pool(name="ps3", bufs=2, space="PSUM") as ppool3,
        ):
            id_sb = cpool.tile([P, P], F16, tag="ident")
            nc.sync.dma_start(id_sb[:, :], id_in[:, :])
            at_sb = cpool.tile([P, G * DCHUNKS], F16, tag="at")
            nc.sync.dma_start(at_sb[:, :], at_in[:, :])
            iota_sb = cpool.tile([P, G], F16, tag="iota")
            nc.sync.dma_start(iota_sb[:, :], iota_in[:, :])
            mask_sb = cpool.tile([P, NB], F16, tag="mask16")
            nc.sync.dma_start(mask_sb[:, :], mask_in[:, :])
            adj_sb = cpool.tile([P, P], F16, tag="bagadj")
            nc.sync.dma_start(adj_sb[:, :], adj_in[:, :])
            q_sb = cpool.tile([P, 3 * n_tiles], F32, tag="q")
            nc.sync.dma_start(q_sb[:, :], q_in[:, :])

            # logit tiles live in a manual ring: column 3 is zeroed once and
            # never rewritten, so exp() can process [128, 4] wholesale (the
            # 4th lane keeps e=1, feeding the unused s_sent column).
            NRING = 3
            logit_ring = []
            for k in range(NRING):
                lt = cpool.tile([P, 4], F32, tag=f"logit{k}")
                nc.vector.memset(lt[:, 3:4], 0.0)
                logit_ring.append(lt)

            def stageL(t):
                xz = pxz.tile([P, D], F16, tag="xz")
                nc.sync.dma_start(xz[:, :], x_in[t * P : (t + 1) * P, :])
                if dbl == "DMAIN":
                    nc.sync.dma_start(xz[:, :], x_in[t * P : (t + 1) * P, :])
                return xz

            def stageA(t, xz):
                xtp = ppool1.tile([P, D], F16, tag="xtp")
                for rr in range(2 if dbl == "T" else 1):
                    for j in range(6):
                        nc.tensor.transpose(
                            xtp[:, j * P : (j + 1) * P],
                            xz[:, j * P : (j + 1) * P],
                            id_sb[:, :],
                        )
                xts = pxts.tile([P, D], F16, tag="xts")
                for rr in range(2 if dbl == "COPY" else 1):
                    if SPLIT > 0:
                        nc.vector.tensor_copy(xts[:, 0:SPLIT], xtp[:, 0:SPLIT])
                    if SPLIT < D:
                        nc.scalar.copy(xts[:, SPLIT:D], xtp[:, SPLIT:D])
                return xts

            def stageB1(t, xts):
                yp = ppool2.tile([P, G], F32, tag="yp")
                nreps = 2 if dbl == "Y" else 1
                for rr in range(nreps):
                    for j in range(6):
                        nc.tensor.matmul(
                            yp[:, :],
                            xts[:, j * P : (j + 1) * P],
                            at_sb[:, j * G : (j + 1) * G],
                            start=(rr == 0 and j == 0),
                            stop=(rr == nreps - 1 and j == 5),
                        )
                # f16 SBUF copy of y: the extraction ops then run all-16-bit
                # on SBUF operands (DVE 2x mode) or on the idle GpSimd.
                ysb = pool.tile([P, G], F16, tag="ysb")
                for rr in range(2 if dbl == "YSB" else 1):
                    nc.scalar.copy(ysb[:, :], yp[:, :])
                logit = logit_ring[t % NRING]
                scratch = pool.tile([P, G], F16, tag="scratch")
                steng = nc.gpsimd if STTENG == "pool" else nc.vector
                for rr in range(2 if dbl == "STT" else 1):
                    for layer in range(3):
                        steng.scalar_tensor_tensor(
                            scratch[:, :],
                            iota_sb[:, :],
                            q_sb[:, 3 * t + layer : 3 * t + layer + 1],
                            ysb[:, :],
                            op0=eq,
                            op1=mult,
                            accum_out=logit[:, layer : layer + 1],
                        )
                return logit

            def stageB2a_exp(t, logit):
                e4 = pool.tile([P, 4], F16, tag="e4")
                nc.scalar.activation(
                    e4[:, :], logit[:, :], mybir.ActivationFunctionType.Exp
                )
                return e4

            def stageB2a_sadj(t, e4):
                # emitted LAST in PE program order so the wait on exp(t)
                # lands after this iteration's transposes, not mid-stream
                ss = ppool_s.tile([P, 4], F32, tag="ss")
                nc.tensor.matmul(
                    ss[:, :], adj_sb[:, :], e4[:, :], start=True, stop=True
                )
                return ss

            def stageB2b(t, e4, ss):
                en = pool.tile([P, 4], F32, tag="en")
                if DIVMODE == "div":
                    nc.vector.tensor_tensor(
                        en[:, :], e4[:, :], ss[:, :], op=div
                    )
                else:
                    rs = pool.tile([P, 4], F32, tag="rs")
                    nc.vector.reciprocal(rs[:, :], ss[:, :])
                    nc.vector.tensor_mul(en[:, :], e4[:, :], rs[:, :])
                wb = pmid.tile([P, NL3], F16, tag="wb")
                for rr in range(2 if dbl == "WB" else 1):
                    for layer in range(3):
                        nc.vector.tensor_scalar_mul(
                            wb[:, layer * NB : (layer + 1) * NB],
                            mask_sb[:, :],
                            en[:, layer : layer + 1],
                        )
                return wb

            def stageC1(t, xz, wb):
                p2 = ppool3.tile([P, OUTW], F32, tag="p2")
                nreps = 2 if dbl == "P2" else 1
                for rr in range(nreps):
                    for j in range(6):
                        nc.tensor.matmul(
                            p2[:, j * NL3 : (j + 1) * NL3],
                            xz[:, j * P : (j + 1) * P],
                            wb[:, :],
                            start=(rr == 0),
                            stop=(rr == nreps - 1),
                        )
                return p2

            outs_group = {}

            def stageC2(t, p2):
                g, k = divmod(t, GROUP)
                if k == 0:
                    outs = pouts.tile(
                        [P, GROUP * OUTW], F16, tag="outs", name="outs"
                    )
                    outs_group[g] = outs
                outs = outs_group[g]
                lo = k * OUTW
                for rr in range(2 if dbl == "C2" else 1):
                    if CSPL > 0:
                        nc.vector.tensor_copy(
                            outs[:, lo : lo + CSPL], p2[:, 0:CSPL]
                        )
                    if CSPL < OUTW:
                        nc.scalar.copy(
                            outs[:, lo + CSPL : lo + OUTW], p2[:, CSPL:OUTW]
                        )
                if k == GROUP - 1:
                    outs = outs_group.pop(g)
                    nc.scalar.dma_start(
                        out[g * P : (g + 1) * P, :], outs[:, :]
                    )
                    if dbl == "DMAOUT":
                        nc.scalar.dma_start(
                            out[g * P : (g + 1) * P, :], outs[:, :]
                        )

            for rep in range(passes):
                stL = {}
                stXts = {}
                stLog = {}
                stE = {}
                stSS = {}
                stWb = {}
                stP2 = {}
                for i in range(-4, n_tiles + 9):
                    if 0 <= i + 4 < n_tiles:
                        stL[i + 4] = stageL(i + 4)
                    if 0 <= i - 8 < n_tiles:
                        stageC2(i - 8, stP2.pop(i - 8))
                    if 0 <= i - 7 < n_tiles:
                        stP2[i - 7] = stageC1(
                            i - 7, stL.pop(i - 7), stWb.pop(i - 7)
                        )
                    if 0 <= i - 5 < n_tiles:
                        stWb[i - 5] = stageB2b(
                            i - 5, stE.pop(i - 5), stSS.pop(i - 5)
                        )
                    if 0 <= i - 4 < n_tiles:
                        stE[i - 4] = stageB2a_exp(i - 4, stLog.pop(i - 4))
                    if 0 <= i - 3 < n_tiles:
                        stLog[i - 3] = stageB1(i - 3, stXts.pop(i - 3))
                    if 0 <= i < n_tiles:
                        stXts[i] = stageA(i, stL[i])
                    if 0 <= i - 4 < n_tiles:
                        stSS[i - 4] = stageB2a_sadj(i - 4, stE[i - 4])

    return nc


# ---------------------------------------------------------------------------
# Host-side constants + sharding
# ---------------------------------------------------------------------------


def _host_constants(attention_matrix):
    a = np.ascontiguousarray(np.asarray(attention_matrix, dtype=np.float32))
    assert a.shape == (G, D)
    at = a.T  # [768, 130]
    at_r = np.ascontiguousarray(
        at.reshape(DCHUNKS, P, G).transpose(1, 0, 2).reshape(P, DCHUNKS * G)
    ).astype(np.float16)
    ident = np.eye(P, dtype=np.float16)
    iota = np.tile(np.arange(G, dtype=np.float16), (P, 1))
    mask16 = (
        (np.arange(P)[:, None] // SEG) == np.arange(NB)[None, :]
    ).astype(np.float16)
    bagadj = (
        (np.arange(P)[:, None] // SEG) == (np.arange(P)[None, :] // SEG)
    ).astype(np.float16)
    return at_r, ident, iota, mask16, bagadj


def make_in_maps(x, attention_query, attention_matrix):
    at_r, ident, iota, mask16, bagadj = _host_constants(attention_matrix)

    x_pad = np.zeros((N_PAD, D), dtype=np.float16)
    x_pad[:N] = x.astype(np.float16)
    q_pad = np.zeros((N_PAD, 3), dtype=np.float32)
    q_pad[:N] = attention_query.astype(np.float32)

    in_maps = []
    for c in range(NCORES):
        xs = x_pad[c * ROWS_CORE : (c + 1) * ROWS_CORE]
        qs = (
            q_pad[c * ROWS_CORE : (c + 1) * ROWS_CORE]
            .reshape(NT, P, 3)
            .transpose(1, 0, 2)
            .reshape(P, 3 * NT)
        )
        in_maps.append(
            {
                "x": np.ascontiguousarray(xs),
                "q": np.ascontiguousarray(qs),
                "at": at_r,
                "ident": ident,
                "iota": iota,
                "mask16": mask16,
                "bagadj": bagadj,
            }
        )
    return in_maps


def kernel(x, attention_query, scope, attention_matrix):
    x = np.asarray(x)
    attention_query = np.asarray(attention_query)
    assert x.shape == (N, D) and attention_query.shape == (N, 3)

    in_maps = make_in_maps(x, attention_query, attention_matrix)

    nc = build_nc()
    trace = bool(int(os.environ.get("KERNEL_TRACE", "0")))
    res = run_bass_kernel_spmd(
        nc, in_maps, core_ids=list(range(NCORES)), trace=trace
    )
    global LAST_EXEC_TIME_NS, LAST_TRACE_PATH
    LAST_EXEC_TIME_NS = res.exec_time_ns
    if trace:
        print(f"HW exec time: {res.exec_time_ns} ns")
        if res.instructions_and_trace is not None:
            LAST_TRACE_PATH = res.instructions_and_trace[1]
            print("trace:", LAST_TRACE_PATH)

    # per-core out is [(NT/G)*128, G*288] fp16: group g, partition p, col
    # k*288 + 48j + 16l + b = out[l, 16(gG+k) + b, 128j + p]
    GROUP = int(os.environ.get("KERNEL_OGROUP", "2"))
    parts = [
        r["out"]
        .reshape(NT // GROUP, P, GROUP, DCHUNKS, 3, NB)
        .transpose(4, 0, 2, 5, 3, 1)
        .reshape(3, BAGS_CORE, D)
        for r in res.results
    ]
    full = np.concatenate(parts, axis=1)[:, :B, :].astype(np.float32)
    return np.ascontiguousarray(full)
